# revision 3
# baseline (speedup 1.0000x reference)
"""Trainium2 Bass kernel for nn_CrossAttention (LN -> Q/K/V proj -> per-position
per-head dot-product gate, no softmax).

Strategy (v2, bf16 + DMA-xbar transpose):
  - Data-parallel over batch: 8 cores x 2 batches each (4096 token rows/core).
  - Host folds LayerNorm affine params into projection weights and converts
    activations + weights to bf16 (tolerance is 2e-2; bf16 keeps us ~5e-3).
  - Per 128-token chunk on device:
      DVE: bn_stats/bn_aggr stats for x and xf, reciprocal, normalize
           (tensor_scalar, bf16 4x mode), q*k product + per-head reduce.
      ACT: sqrt(var+eps), PSUM->SBUF drains of q/k/v as bf16.
      DMA xbar: SBUF->SBUF transpose of normalized activations (replaces the
           PE transposes + PSUM round-trip of v1).
      PE:  16 bf16 matmuls (N=512) per chunk -- this is the bottleneck engine,
           kept free of transposes.
      Pool(gpsimd): y1/y2 broadcast-gate multiplies; also issues input DMAs
           (cheap sequencer path).
      SP:  issues the 2 transposes + fused y-store.
  - y1|y2 stored as one fused [NTOK, 1024] bf16 tensor (1 DMA per chunk).
"""

import math
from contextlib import ExitStack

import numpy as np
import ml_dtypes

import concourse.bacc as bacc
import concourse.bass as bass
import concourse.tile as tile
from concourse import mybir
from concourse.bass_utils import run_bass_kernel_spmd

F32 = mybir.dt.float32
BF16 = mybir.dt.bfloat16
AF = mybir.ActivationFunctionType
ALU = mybir.AluOpType

# Problem shapes (hardcoded per spec)
B, T, D, L, HD = 16, 2048, 512, 768, 512
H, DH = 8, 64
EPS = 1e-5
NCORES = 8
B_LOC = B // NCORES          # 2
NTOK = B_LOC * T             # 4096 token rows per core
P = 128
NCHUNK = NTOK // P           # 32
DC = D // P                  # 4 contraction chunks for x
LC = L // P                  # 6 contraction chunks for xf

BF16_NP = ml_dtypes.bfloat16


def build_program(with_bias: bool):
    nc = bacc.Bacc(
        "TRN2",
        target_bir_lowering=False,
        debug=False,
        enable_asserts=False,
        num_devices=NCORES,
    )

    x_d = nc.dram_tensor("x", [NTOK, D], BF16, kind="ExternalInput").ap()
    xf_d = nc.dram_tensor("xf", [NTOK, L], BF16, kind="ExternalInput").ap()
    wq_d = nc.dram_tensor("wq", [P, DC, HD], BF16, kind="ExternalInput").ap()
    wk_d = nc.dram_tensor("wk", [P, LC, HD], BF16, kind="ExternalInput").ap()
    wv_d = nc.dram_tensor("wv", [P, LC, HD], BF16, kind="ExternalInput").ap()
    if with_bias:
        bq_d = nc.dram_tensor("bq", [1, HD], BF16, kind="ExternalInput").ap()
        bk_d = nc.dram_tensor("bk", [1, HD], BF16, kind="ExternalInput").ap()
        bv_d = nc.dram_tensor("bv", [1, HD], BF16, kind="ExternalInput").ap()
    y_d = nc.dram_tensor("y", [NTOK, 2 * HD], BF16, kind="ExternalOutput").ap()

    with tile.TileContext(nc) as tc, ExitStack() as ctx:
        consts = ctx.enter_context(tc.tile_pool(name="consts", bufs=1))
        loads = ctx.enter_context(tc.tile_pool(name="loads", bufs=4))
        mids = ctx.enter_context(tc.tile_pool(name="mids", bufs=3))
        small = ctx.enter_context(tc.tile_pool(name="small", bufs=4))
        outs = ctx.enter_context(tc.tile_pool(name="outs", bufs=3))
        gp = ctx.enter_context(tc.tile_pool(name="gp", bufs=6, space="PSUM"))

        # Resident constants
        wq_s = consts.tile([P, DC, HD], BF16)
        nc.sync.dma_start(out=wq_s, in_=wq_d)
        wk_s = consts.tile([P, LC, HD], BF16)
        nc.sync.dma_start(out=wk_s, in_=wk_d)
        wv_s = consts.tile([P, LC, HD], BF16)
        nc.sync.dma_start(out=wv_s, in_=wv_d)
        eps_t = consts.tile([P, 1], F32)
        nc.vector.memset(eps_t, EPS)
        if with_bias:
            ones_row = consts.tile([1, P], BF16)
            nc.vector.memset(ones_row, 1.0)
            bq_s = consts.tile([1, HD], BF16)
            nc.sync.dma_start(out=bq_s, in_=bq_d)
            bk_s = consts.tile([1, HD], BF16)
            nc.sync.dma_start(out=bk_s, in_=bk_d)
            bv_s = consts.tile([1, HD], BF16)
            nc.sync.dma_start(out=bv_s, in_=bv_d)

        for i in range(NCHUNK):
            rows = bass.ts(i, P)

            x_t = loads.tile([P, D], BF16, tag="x_t")
            nc.gpsimd.dma_start(out=x_t, in_=x_d[rows, :])
            xf_t = loads.tile([P, L], BF16, tag="xf_t")
            nc.gpsimd.dma_start(out=xf_t, in_=xf_d[rows, :])

            # ---- stats on DVE (bn_stats / bn_aggr) ----
            st_x = small.tile([P, 6], F32, tag="st_x")
            nc.vector.bn_stats(st_x, x_t)
            mv_x = small.tile([P, 2], F32, tag="mv_x")
            nc.vector.bn_aggr(mv_x, st_x)
            st_f = small.tile([P, 2, 6], F32, tag="st_f")
            nc.vector.bn_stats(st_f[:, 0, :], xf_t[:, 0 : L // 2])
            nc.vector.bn_stats(st_f[:, 1, :], xf_t[:, L // 2 : L])
            mv_f = small.tile([P, 2], F32, tag="mv_f")
            nc.vector.bn_aggr(mv_f, st_f)

            negmx = small.tile([P, 1], F32, tag="negmx")
            nc.vector.tensor_scalar_mul(negmx, mv_x[:, 0:1], -1.0)
            negmf = small.tile([P, 1], F32, tag="negmf")
            nc.vector.tensor_scalar_mul(negmf, mv_f[:, 0:1], -1.0)
            sigx = small.tile([P, 1], F32, tag="sigx")
            nc.scalar.activation(sigx, mv_x[:, 1:2], AF.Sqrt, bias=eps_t, scale=1.0)
            sigf = small.tile([P, 1], F32, tag="sigf")
            nc.scalar.activation(sigf, mv_f[:, 1:2], AF.Sqrt, bias=eps_t, scale=1.0)
            rsx = small.tile([P, 1], F32, tag="rsx")
            nc.vector.reciprocal(rsx, sigx)
            rsf = small.tile([P, 1], F32, tag="rsf")
            nc.vector.reciprocal(rsf, sigf)

            # ---- normalize on DVE (bf16 tensor_scalar, 4x mode) ----
            xh = mids.tile([P, D], BF16, tag="xh")
            nc.vector.tensor_scalar(
                out=xh, in0=x_t, scalar1=negmx, scalar2=rsx, op0=ALU.add, op1=ALU.mult
            )
            xfh = mids.tile([P, L], BF16, tag="xfh")
            nc.vector.tensor_scalar(
                out=xfh, in0=xf_t, scalar1=negmf, scalar2=rsf, op0=ALU.add, op1=ALU.mult
            )

            # ---- transpose via DMA xbar (SBUF->SBUF), issued from SP ----
            xhT = mids.tile([P, DC, P], BF16, tag="xhT")
            nc.sync.dma_start(out=xhT, in_=xh, transpose=True)
            xfhT = mids.tile([P, LC, P], BF16, tag="xfhT")
            nc.sync.dma_start(out=xfhT, in_=xfh, transpose=True)

            # ---- projections (bf16 matmuls; q side pre-scaled by 1/8) ----
            gq = gp.tile([P, HD], F32, tag="g")
            for c in range(DC):
                nc.tensor.matmul(
                    gq,
                    lhsT=xhT[:, c, :],
                    rhs=wq_s[:, c, :],
                    start=(c == 0),
                    stop=(c == DC - 1 and not with_bias),
                )
            if with_bias:
                nc.tensor.matmul(gq, lhsT=ones_row, rhs=bq_s, start=False, stop=True)
            gk = gp.tile([P, HD], F32, tag="g")
            gv = gp.tile([P, HD], F32, tag="g")
            for c in range(LC):
                nc.tensor.matmul(
                    gk,
                    lhsT=xfhT[:, c, :],
                    rhs=wk_s[:, c, :],
                    start=(c == 0),
                    stop=(c == LC - 1 and not with_bias),
                )
                nc.tensor.matmul(
                    gv,
                    lhsT=xfhT[:, c, :],
                    rhs=wv_s[:, c, :],
                    start=(c == 0),
                    stop=(c == LC - 1 and not with_bias),
                )
            if with_bias:
                nc.tensor.matmul(gk, lhsT=ones_row, rhs=bk_s, start=False, stop=True)
                nc.tensor.matmul(gv, lhsT=ones_row, rhs=bv_s, start=False, stop=True)

            # ---- drain q/k/v PSUM -> SBUF as bf16 (ACT) ----
            qs = mids.tile([P, HD], BF16, tag="qs")
            nc.scalar.copy(qs, gq)
            ks = mids.tile([P, HD], BF16, tag="ks")
            nc.scalar.copy(ks, gk)
            vs = mids.tile([P, HD], BF16, tag="vs")
            nc.scalar.copy(vs, gv)

            # ---- gate: w[t,h] = sum_d qs[t,h,d]*ks[t,h,d]  (qs is q/8) ----
            pp = mids.tile([P, HD], BF16, tag="pp")
            nc.vector.tensor_tensor(out=pp, in0=qs, in1=ks, op=ALU.mult)
            w = small.tile([P, H], F32, tag="w")
            nc.vector.tensor_reduce(
                out=w,
                in_=pp.rearrange("p (h d) -> p h d", h=H),
                axis=mybir.AxisListType.X,
                op=ALU.add,
            )
            g1 = small.tile([P, H], F32, tag="g1")
            nc.vector.tensor_scalar(
                out=g1, in0=w, scalar1=-8.0, scalar2=8.0, op0=ALU.mult, op1=ALU.add
            )
            # free-dim step-0 broadcast reads of w/g1 across each head's lanes
            w_bcast = bass.AP(
                tensor=w.tensor, offset=w.offset, ap=[w.ap[0], w.ap[1], [0, DH]]
            )
            g1_bcast = bass.AP(
                tensor=g1.tensor, offset=g1.offset, ap=[g1.ap[0], g1.ap[1], [0, DH]]
            )

            # ---- y1 = (8-8w)*qs ; y2 = w*vs  (Pool engine, SBUF only) ----
            yt = outs.tile([P, 2, HD], BF16, tag="yt")
            nc.gpsimd.tensor_tensor(
                out=yt[:, 0, :].rearrange("p (h d) -> p h d", h=H),
                in0=g1_bcast,
                in1=qs.rearrange("p (h d) -> p h d", h=H),
                op=ALU.mult,
            )
            nc.gpsimd.tensor_tensor(
                out=yt[:, 1, :].rearrange("p (h d) -> p h d", h=H),
                in0=w_bcast,
                in1=vs.rearrange("p (h d) -> p h d", h=H),
                op=ALU.mult,
            )

            nc.sync.dma_start(out=y_d[rows, :], in_=yt)

    nc.compile()
    return nc


_PROGRAM_CACHE: dict = {}


def _get_program(with_bias: bool):
    if with_bias not in _PROGRAM_CACHE:
        _PROGRAM_CACHE[with_bias] = build_program(with_bias)
    return _PROGRAM_CACHE[with_bias]


def _prep_host(inputs):
    norm_w = np.asarray(inputs["norm_w"], np.float32)
    norm_b = np.asarray(inputs["norm_b"], np.float32)
    tnorm_w = np.asarray(inputs["tnorm_w"], np.float32)
    tnorm_b = np.asarray(inputs["tnorm_b"], np.float32)
    Wq = np.asarray(inputs["Wq"], np.float32)
    Wk = np.asarray(inputs["Wk"], np.float32)
    Wv = np.asarray(inputs["Wv"], np.float32)

    scale_q = 1.0 / math.sqrt(DH)
    wq_eff = (norm_w[:, None] * Wq.T) * scale_q      # [D, HD]
    wk_eff = tnorm_w[:, None] * Wk.T                 # [L, HD]
    wv_eff = tnorm_w[:, None] * Wv.T                 # [L, HD]
    bq = (norm_b @ Wq.T) * scale_q                   # [HD]
    bk = tnorm_b @ Wk.T
    bv = tnorm_b @ Wv.T

    # [D, HD] -> [P, DC, HD]: partition p holds rows {c*128+p}
    wq_h = np.ascontiguousarray(
        wq_eff.reshape(DC, P, HD).transpose(1, 0, 2)
    ).astype(BF16_NP)
    wk_h = np.ascontiguousarray(
        wk_eff.reshape(LC, P, HD).transpose(1, 0, 2)
    ).astype(BF16_NP)
    wv_h = np.ascontiguousarray(
        wv_eff.reshape(LC, P, HD).transpose(1, 0, 2)
    ).astype(BF16_NP)
    with_bias = bool(np.any(norm_b) or np.any(tnorm_b))
    return wq_h, wk_h, wv_h, bq, bk, bv, with_bias


def make_in_maps(inputs):
    x = np.asarray(inputs["x"], np.float32)
    xf = np.asarray(inputs["xf"], np.float32)
    wq_h, wk_h, wv_h, bq, bk, bv, with_bias = _prep_host(inputs)
    x_b = x.astype(BF16_NP)
    xf_b = xf.astype(BF16_NP)

    in_maps = []
    for i in range(NCORES):
        m = {
            "x": np.ascontiguousarray(
                x_b[i * B_LOC : (i + 1) * B_LOC].reshape(NTOK, D)
            ),
            "xf": np.ascontiguousarray(
                xf_b[i * B_LOC : (i + 1) * B_LOC].reshape(NTOK, L)
            ),
            "wq": wq_h,
            "wk": wk_h,
            "wv": wv_h,
        }
        if with_bias:
            m["bq"] = bq.reshape(1, HD).astype(BF16_NP)
            m["bk"] = bk.reshape(1, HD).astype(BF16_NP)
            m["bv"] = bv.reshape(1, HD).astype(BF16_NP)
        in_maps.append(m)
    return in_maps, with_bias


def split_y(y_flat):
    """[NTOK, 2*HD] fused output -> (y1, y2) each [B_LOC, T, HD] f32."""
    y = np.asarray(y_flat).reshape(B_LOC, T, 2, HD).astype(np.float32)
    return y[:, :, 0, :], y[:, :, 1, :]


def kernel(**inputs):
    in_maps, with_bias = make_in_maps(inputs)
    nc = _get_program(with_bias)
    res = run_bass_kernel_spmd(nc, in_maps, core_ids=list(range(NCORES)))
    y1_parts = []
    y2_parts = []
    for r in res.results:
        y1_c, y2_c = split_y(r["y"])
        y1_parts.append(y1_c)
        y2_parts.append(y2_c)
    return (np.concatenate(y1_parts, axis=0), np.concatenate(y2_parts, axis=0))


# revision 4
# speedup vs baseline: 1.3912x; 1.3912x over previous
"""Trainium2 Bass kernel for nn_CrossAttention (LN -> Q/K/V proj -> per-position
per-head dot-product gate, no softmax).

Strategy (v3, bf16 + DMA-xbar transpose + explicit software pipeline):
  - Data-parallel over batch: 8 cores x 2 batches each (4096 token rows/core).
  - Host folds LayerNorm affine params into projection weights, converts
    activations + weights to bf16, and packs x|xf into one [NTOK, 1280]
    tensor so each chunk needs ONE load DMA and ONE xbar transpose.
  - Per 128-token chunk the stages are:
      S0  SP:   DMA load xx chunk
      S1a DVE:  bn_stats/bn_aggr for x and xf
      S1b ACT:  sqrt(var+eps);  DVE: reciprocal, normalize (x-m)*rs -> bf16
      S2  SP:   DMA xbar transpose (SBUF->SBUF) -> [128, 10, 128]
      S3  PE:   16 bf16 matmuls N=512 (q: 4 chunks, k/v interleaved: 12)
      S4  ACT:  drain gq -> qs, gkv -> kvs (fused 2-bank read) as bf16
      S5  Pool: pp = qs*ks;  DVE: w = per-head reduce, g1 = 8-8w;
          Pool: y1 = g1*qs, y2 = w*vs (broadcast APs)
      S6  ACT:  DMA store fused y1|y2 row
  - Stages are emitted in skewed "waves" so every engine queue sees its ops
    in dependency-arrival order (no head-of-line blocking), keeping the PE
    (the bottleneck at ~3.4us/chunk of bf16 matmul) continuously fed.
"""

import math
from contextlib import ExitStack

import numpy as np
import ml_dtypes

import concourse.bacc as bacc
import concourse.bass as bass
import concourse.tile as tile
from concourse import mybir
from concourse.bass_utils import run_bass_kernel_spmd

F32 = mybir.dt.float32
BF16 = mybir.dt.bfloat16
AF = mybir.ActivationFunctionType
ALU = mybir.AluOpType

# Problem shapes (hardcoded per spec)
B, T, D, L, HD = 16, 2048, 512, 768, 512
H, DH = 8, 64
EPS = 1e-5
NCORES = 8
B_LOC = B // NCORES          # 2
NTOK = B_LOC * T             # 4096 token rows per core
P = 128
NCHUNK = NTOK // P           # 32
DC = D // P                  # 4 contraction chunks for x
LC = L // P                  # 6 contraction chunks for xf
W_ALL = D + L                # 1280 packed width
CC = W_ALL // P              # 10 transposed chunks

BF16_NP = ml_dtypes.bfloat16


def build_program(with_bias: bool):
    nc = bacc.Bacc(
        "TRN2",
        target_bir_lowering=False,
        debug=False,
        enable_asserts=False,
        num_devices=NCORES,
    )

    xx_d = nc.dram_tensor("xx", [NTOK, W_ALL], BF16, kind="ExternalInput").ap()
    wq_d = nc.dram_tensor("wq", [P, DC, HD], BF16, kind="ExternalInput").ap()
    wk_d = nc.dram_tensor("wk", [P, LC, HD], BF16, kind="ExternalInput").ap()
    wv_d = nc.dram_tensor("wv", [P, LC, HD], BF16, kind="ExternalInput").ap()
    if with_bias:
        bq_d = nc.dram_tensor("bq", [1, HD], BF16, kind="ExternalInput").ap()
        bk_d = nc.dram_tensor("bk", [1, HD], BF16, kind="ExternalInput").ap()
        bv_d = nc.dram_tensor("bv", [1, HD], BF16, kind="ExternalInput").ap()
    y_d = nc.dram_tensor("y", [NTOK, 2 * HD], BF16, kind="ExternalOutput").ap()

    with tile.TileContext(nc) as tc, ExitStack() as ctx:
        consts = ctx.enter_context(tc.tile_pool(name="consts", bufs=1))
        loads = ctx.enter_context(tc.tile_pool(name="loads", bufs=4))
        mids = ctx.enter_context(tc.tile_pool(name="mids", bufs=4))
        small = ctx.enter_context(tc.tile_pool(name="small", bufs=4))
        outs = ctx.enter_context(tc.tile_pool(name="outs", bufs=4))
        gpq = ctx.enter_context(tc.tile_pool(name="gpq", bufs=2, space="PSUM"))
        gpkv = ctx.enter_context(tc.tile_pool(name="gpkv", bufs=2, space="PSUM"))

        # Resident constants
        wq_s = consts.tile([P, DC, HD], BF16)
        nc.sync.dma_start(out=wq_s, in_=wq_d)
        wk_s = consts.tile([P, LC, HD], BF16)
        nc.sync.dma_start(out=wk_s, in_=wk_d)
        wv_s = consts.tile([P, LC, HD], BF16)
        nc.sync.dma_start(out=wv_s, in_=wv_d)
        eps_t = consts.tile([P, 1], F32)
        nc.vector.memset(eps_t, EPS)
        if with_bias:
            ones_row = consts.tile([1, P], BF16)
            nc.vector.memset(ones_row, 1.0)
            bq_s = consts.tile([1, HD], BF16)
            nc.sync.dma_start(out=bq_s, in_=bq_d)
            bk_s = consts.tile([1, HD], BF16)
            nc.sync.dma_start(out=bk_s, in_=bk_d)
            bv_s = consts.tile([1, HD], BF16)
            nc.sync.dma_start(out=bv_s, in_=bv_d)

        tk: dict = {}

        def s0_load(i):
            xx_t = loads.tile([P, W_ALL], BF16, tag="xx_t")
            tk["xx", i] = xx_t
            nc.sync.dma_start(out=xx_t, in_=xx_d[bass.ts(i, P), :])

        def s1a_stats(i):
            xx_t = tk["xx", i]
            st_x = small.tile([P, 6], F32, tag="st_x")
            nc.vector.bn_stats(st_x, xx_t[:, 0:D])
            mv_x = small.tile([P, 2], F32, tag="mv_x")
            tk["mv_x", i] = mv_x
            nc.vector.bn_aggr(mv_x, st_x)
            st_f = small.tile([P, 2, 6], F32, tag="st_f")
            nc.vector.bn_stats(st_f[:, 0, :], xx_t[:, D : D + L // 2])
            nc.vector.bn_stats(st_f[:, 1, :], xx_t[:, D + L // 2 : W_ALL])
            mv_f = small.tile([P, 2], F32, tag="mv_f")
            tk["mv_f", i] = mv_f
            nc.vector.bn_aggr(mv_f, st_f)

        def s1b_norm(i):
            xx_t = tk["xx", i]
            mv_x = tk["mv_x", i]
            mv_f = tk["mv_f", i]
            sig2 = small.tile([P, 2], F32, tag="sig2")
            nc.scalar.activation(
                sig2[:, 0:1], mv_x[:, 1:2], AF.Sqrt, bias=eps_t, scale=1.0
            )
            nc.scalar.activation(
                sig2[:, 1:2], mv_f[:, 1:2], AF.Sqrt, bias=eps_t, scale=1.0
            )
            rs2 = small.tile([P, 2], F32, tag="rs2")
            nc.vector.reciprocal(rs2, sig2)
            xhh = mids.tile([P, W_ALL], BF16, tag="xhh")
            tk["xhh", i] = xhh
            nc.vector.tensor_scalar(
                out=xhh[:, 0:D],
                in0=xx_t[:, 0:D],
                scalar1=mv_x[:, 0:1],
                scalar2=rs2[:, 0:1],
                op0=ALU.subtract,
                op1=ALU.mult,
            )
            nc.vector.tensor_scalar(
                out=xhh[:, D:W_ALL],
                in0=xx_t[:, D:W_ALL],
                scalar1=mv_f[:, 0:1],
                scalar2=rs2[:, 1:2],
                op0=ALU.subtract,
                op1=ALU.mult,
            )

        def s2_transpose(i):
            xhh = tk["xhh", i]
            xxT = mids.tile([P, CC, P], BF16, tag="xxT")
            tk["xxT", i] = xxT
            nc.sync.dma_start(out=xxT, in_=xhh, transpose=True)

        def s3_matmul(i):
            xxT = tk["xxT", i]
            gq = gpq.tile([P, HD], F32, tag="gq")
            tk["gq", i] = gq
            for c in range(DC):
                nc.tensor.matmul(
                    gq,
                    lhsT=xxT[:, c, :],
                    rhs=wq_s[:, c, :],
                    start=(c == 0),
                    stop=(c == DC - 1 and not with_bias),
                )
            if with_bias:
                nc.tensor.matmul(gq, lhsT=ones_row, rhs=bq_s, start=False, stop=True)
            gkv = gpkv.tile([P, 2, HD], F32, tag="gkv")
            tk["gkv", i] = gkv
            for c in range(LC):
                nc.tensor.matmul(
                    gkv[:, 0, :],
                    lhsT=xxT[:, DC + c, :],
                    rhs=wk_s[:, c, :],
                    start=(c == 0),
                    stop=(c == LC - 1 and not with_bias),
                )
                nc.tensor.matmul(
                    gkv[:, 1, :],
                    lhsT=xxT[:, DC + c, :],
                    rhs=wv_s[:, c, :],
                    start=(c == 0),
                    stop=(c == LC - 1 and not with_bias),
                )
            if with_bias:
                nc.tensor.matmul(
                    gkv[:, 0, :], lhsT=ones_row, rhs=bk_s, start=False, stop=True
                )
                nc.tensor.matmul(
                    gkv[:, 1, :], lhsT=ones_row, rhs=bv_s, start=False, stop=True
                )

        def s4_drain(i):
            qs = mids.tile([P, HD], BF16, tag="qs")
            tk["qs", i] = qs
            nc.scalar.copy(qs, tk["gq", i])
            kvs = mids.tile([P, 2, HD], BF16, tag="kvs")
            tk["kvs", i] = kvs
            nc.scalar.copy(kvs, tk["gkv", i])

        def s5_gate(i):
            qs = tk["qs", i]
            kvs = tk["kvs", i]
            pp = mids.tile([P, HD], BF16, tag="pp")
            nc.gpsimd.tensor_tensor(out=pp, in0=qs, in1=kvs[:, 0, :], op=ALU.mult)
            w = small.tile([P, H], F32, tag="w")
            nc.vector.tensor_reduce(
                out=w,
                in_=pp.rearrange("p (h d) -> p h d", h=H),
                axis=mybir.AxisListType.X,
                op=ALU.add,
            )
            g1 = small.tile([P, H], F32, tag="g1")
            nc.vector.tensor_scalar(
                out=g1, in0=w, scalar1=-8.0, scalar2=8.0, op0=ALU.mult, op1=ALU.add
            )
            # free-dim step-0 broadcast reads of w/g1 across each head's lanes
            w_bcast = bass.AP(
                tensor=w.tensor, offset=w.offset, ap=[w.ap[0], w.ap[1], [0, DH]]
            )
            g1_bcast = bass.AP(
                tensor=g1.tensor, offset=g1.offset, ap=[g1.ap[0], g1.ap[1], [0, DH]]
            )
            yt = outs.tile([P, 2, HD], BF16, tag="yt")
            tk["yt", i] = yt
            nc.gpsimd.tensor_tensor(
                out=yt[:, 0, :].rearrange("p (h d) -> p h d", h=H),
                in0=g1_bcast,
                in1=qs.rearrange("p (h d) -> p h d", h=H),
                op=ALU.mult,
            )
            nc.gpsimd.tensor_tensor(
                out=yt[:, 1, :].rearrange("p (h d) -> p h d", h=H),
                in0=w_bcast,
                in1=kvs[:, 1, :].rearrange("p (h d) -> p h d", h=H),
                op=ALU.mult,
            )

        def s6_store(i):
            nc.scalar.dma_start(out=y_d[bass.ts(i, P), :], in_=tk["yt", i])

        stages = [s0_load, s1a_stats, s1b_norm, s2_transpose, s3_matmul,
                  s4_drain, s5_gate, s6_store]
        nstage = len(stages)
        for wave in range(NCHUNK + nstage - 1):
            for s, fn in enumerate(stages):
                i = wave - s
                if 0 <= i < NCHUNK:
                    fn(i)

    nc.compile()
    return nc


_PROGRAM_CACHE: dict = {}


def _get_program(with_bias: bool):
    if with_bias not in _PROGRAM_CACHE:
        _PROGRAM_CACHE[with_bias] = build_program(with_bias)
    return _PROGRAM_CACHE[with_bias]


def _prep_host(inputs):
    norm_w = np.asarray(inputs["norm_w"], np.float32)
    norm_b = np.asarray(inputs["norm_b"], np.float32)
    tnorm_w = np.asarray(inputs["tnorm_w"], np.float32)
    tnorm_b = np.asarray(inputs["tnorm_b"], np.float32)
    Wq = np.asarray(inputs["Wq"], np.float32)
    Wk = np.asarray(inputs["Wk"], np.float32)
    Wv = np.asarray(inputs["Wv"], np.float32)

    scale_q = 1.0 / math.sqrt(DH)
    wq_eff = (norm_w[:, None] * Wq.T) * scale_q      # [D, HD]
    wk_eff = tnorm_w[:, None] * Wk.T                 # [L, HD]
    wv_eff = tnorm_w[:, None] * Wv.T                 # [L, HD]
    bq = (norm_b @ Wq.T) * scale_q                   # [HD]
    bk = tnorm_b @ Wk.T
    bv = tnorm_b @ Wv.T

    # [D, HD] -> [P, DC, HD]: partition p holds rows {c*128+p}
    wq_h = np.ascontiguousarray(
        wq_eff.reshape(DC, P, HD).transpose(1, 0, 2)
    ).astype(BF16_NP)
    wk_h = np.ascontiguousarray(
        wk_eff.reshape(LC, P, HD).transpose(1, 0, 2)
    ).astype(BF16_NP)
    wv_h = np.ascontiguousarray(
        wv_eff.reshape(LC, P, HD).transpose(1, 0, 2)
    ).astype(BF16_NP)
    with_bias = bool(np.any(norm_b) or np.any(tnorm_b))
    return wq_h, wk_h, wv_h, bq, bk, bv, with_bias


def make_in_maps(inputs):
    x = np.asarray(inputs["x"], np.float32)
    xf = np.asarray(inputs["xf"], np.float32)
    wq_h, wk_h, wv_h, bq, bk, bv, with_bias = _prep_host(inputs)
    xx = np.concatenate(
        [x.astype(BF16_NP).reshape(B, T, D), xf.astype(BF16_NP).reshape(B, T, L)],
        axis=2,
    )  # [B, T, 1280] bf16

    in_maps = []
    for i in range(NCORES):
        m = {
            "xx": np.ascontiguousarray(
                xx[i * B_LOC : (i + 1) * B_LOC].reshape(NTOK, W_ALL)
            ),
            "wq": wq_h,
            "wk": wk_h,
            "wv": wv_h,
        }
        if with_bias:
            m["bq"] = bq.reshape(1, HD).astype(BF16_NP)
            m["bk"] = bk.reshape(1, HD).astype(BF16_NP)
            m["bv"] = bv.reshape(1, HD).astype(BF16_NP)
        in_maps.append(m)
    return in_maps, with_bias


def split_y(y_flat):
    """[NTOK, 2*HD] fused output -> (y1, y2) each [B_LOC, T, HD] f32."""
    y = np.asarray(y_flat).reshape(B_LOC, T, 2, HD).astype(np.float32)
    return y[:, :, 0, :], y[:, :, 1, :]


def kernel(**inputs):
    in_maps, with_bias = make_in_maps(inputs)
    nc = _get_program(with_bias)
    res = run_bass_kernel_spmd(nc, in_maps, core_ids=list(range(NCORES)))
    y1_parts = []
    y2_parts = []
    for r in res.results:
        y1_c, y2_c = split_y(r["y"])
        y1_parts.append(y1_c)
        y2_parts.append(y2_c)
    return (np.concatenate(y1_parts, axis=0), np.concatenate(y2_parts, axis=0))
